# revision 1
# baseline (speedup 1.0000x reference)
"""InteractionMapInit Trainium2 kernel.

out[i, j, :] = tanh( (X@Wt + bt)[i] - (Dft@Wd + bd)[j] + dnorm[i, j] )  if seg_res[i] == seg_atom[j]
             = 0                                                        otherwise

The (residue, atom) mask is block-diagonal over the B=8 drug-target pairs, so
each of the 8 NeuronCores computes exactly one block (padded to a common shape
for SPMD) and the host scatters the blocks into a zeros output.

Per-core device computation for its block (Rp padded rows, Ap padded atoms):
  tfT   [H, Rp]    = Wt.T @ X.T + bt           (fp32 matmuls)
  df    [Ap, H]    = Dft.T @ Wd + bd           (fp32 matmul)
  D2    [128, Ap]  = |t_i|^2 + |d_j|^2 - 2 t_i.d_j   (K=5 fp32 matmul per row tile)
  D     = sqrt(D2) (ACT), dmin/dmax via DVE reduces + PE transpose
  dnorm = (D - dmin) / (dmax - dmin)
  out[i, (j,h)] = tanh( tf[i,h] - df[j,h] + dnorm[i,j] ) materialized on the PE
  with two fp32r matmuls per 512-wide PSUM chunk:
    mm1: lhsT = tfT row-tile      rhs = I4  (I_128 tiled 4x: delta(h', h))
    mm2: lhsT = [dnorm^T; ones]   rhs = R2D ([delta(j',j) x ones_H ; -df_flat])
  then ACT tanh PSUM -> SBUF and DMA out.

Padding: positions are edge-replicated (keeps per-block min/max distances
exact), features zero-padded; padded rows/cols are discarded on the host.
"""

import numpy as np

NR, NA, TD, DD, H, B = 3200, 320, 512, 128, 128, 8
NCORES = 8
P = 128

_last_results = None  # BassKernelResults of the most recent run (for test harness)


def _host_prep(target_feature, drug_feature, target_pos, drug_pos,
               Wt, bt, Wd, bd, seg_res, seg_atom):
    f32 = np.float32
    X = np.ascontiguousarray(np.asarray(target_feature, f32))
    Dft = np.ascontiguousarray(np.asarray(drug_feature, f32))
    tp = np.asarray(target_pos, f32)
    dp = np.asarray(drug_pos, f32)
    Wt = np.ascontiguousarray(np.asarray(Wt, f32))
    Wd = np.ascontiguousarray(np.asarray(Wd, f32))
    bt = np.asarray(bt, f32).reshape(1, H)
    bd = np.asarray(bd, f32).reshape(1, H)
    seg_res = np.asarray(seg_res)
    seg_atom = np.asarray(seg_atom)

    r0 = np.searchsorted(seg_res, np.arange(B), side="left")
    r1 = np.searchsorted(seg_res, np.arange(B), side="right")
    a0 = np.searchsorted(seg_atom, np.arange(B), side="left")
    a1 = np.searchsorted(seg_atom, np.arange(B), side="right")
    r_cnt = (r1 - r0).astype(int)
    a_cnt = (a1 - a0).astype(int)

    Rp = max(P, int(-(-max(r_cnt) // P)) * P)
    Ap = max(4, int(-(-max(a_cnt) // 4)) * 4)
    assert Ap + 1 <= 128, f"block atom count too large: {max(a_cnt)}"

    AH = Ap * H
    I4 = np.ascontiguousarray(np.tile(np.eye(P, dtype=f32), (1, 4)))
    R2D = np.zeros((Ap + 1, AH), f32)
    R2D[:Ap, :] = np.kron(np.eye(Ap, dtype=f32), np.ones((1, H), f32))

    in_maps = []
    for c in range(B):
        rc, ac = r_cnt[c], a_cnt[c]
        XT = np.zeros((TD, Rp), f32)
        DFT = np.zeros((DD, Ap), f32)
        tpp = np.zeros((Rp, 3), f32)
        dpp = np.zeros((Ap, 3), f32)
        if rc > 0:
            XT[:, :rc] = X[r0[c]:r1[c]].T
            tpp[:rc] = tp[r0[c]:r1[c]]
            tpp[rc:] = tp[r1[c] - 1]
        if ac > 0:
            DFT[:, :ac] = Dft[a0[c]:a1[c]].T
            dpp[:ac] = dp[a0[c]:a1[c]]
            dpp[ac:] = dp[a1[c] - 1]

        LHD = np.zeros((5, Rp), f32)
        LHD[0:3] = tpp.T
        LHD[3] = 1.0
        LHD[4] = (tpp * tpp).sum(axis=1)
        RHD = np.zeros((5, Ap), f32)
        RHD[0:3] = -2.0 * dpp.T
        RHD[3] = (dpp * dpp).sum(axis=1)
        RHD[4] = 1.0

        in_maps.append({
            "xt": np.ascontiguousarray(XT),
            "dft": np.ascontiguousarray(DFT),
            "lhd": np.ascontiguousarray(LHD),
            "rhd": np.ascontiguousarray(RHD),
            "wt": Wt, "wd": Wd, "bt": bt, "bd": bd,
            "i4": I4, "r2d": R2D,
        })

    meta = dict(r0=r0, a0=a0, r_cnt=r_cnt, a_cnt=a_cnt, Rp=Rp, Ap=Ap)
    return in_maps, meta


def build_bass(Rp, Ap):
    from contextlib import ExitStack

    import concourse.bacc as bacc
    import concourse.mybir as mybir
    import concourse.tile as tile
    from concourse.masks import make_identity

    F32 = mybir.dt.float32
    F32R = mybir.dt.float32r
    AX = mybir.AxisListType
    OP = mybir.AluOpType
    AF = mybir.ActivationFunctionType

    K_TD = TD // P        # 4 contraction chunks for the target linear
    RT = Rp // P          # 128-row tiles
    NCH = Ap // 4         # 512-wide psum chunks (4 atoms x H)
    AH = Ap * H

    nc = bacc.Bacc("TRN2", target_bir_lowering=False, debug=False,
                   num_devices=NCORES)

    xt_d = nc.dram_tensor("xt", [TD, Rp], F32, kind="ExternalInput").ap()
    wt_d = nc.dram_tensor("wt", [TD, H], F32, kind="ExternalInput").ap()
    wd_d = nc.dram_tensor("wd", [DD, H], F32, kind="ExternalInput").ap()
    bt_d = nc.dram_tensor("bt", [1, H], F32, kind="ExternalInput").ap()
    bd_d = nc.dram_tensor("bd", [1, H], F32, kind="ExternalInput").ap()
    dft_d = nc.dram_tensor("dft", [DD, Ap], F32, kind="ExternalInput").ap()
    lhd_d = nc.dram_tensor("lhd", [5, Rp], F32, kind="ExternalInput").ap()
    rhd_d = nc.dram_tensor("rhd", [5, Ap], F32, kind="ExternalInput").ap()
    i4_d = nc.dram_tensor("i4", [P, 512], F32R, kind="ExternalInput").ap()
    r2d_d = nc.dram_tensor("r2d", [Ap + 1, AH], F32R, kind="ExternalInput").ap()
    out_d = nc.dram_tensor("out", [Rp, AH], F32, kind="ExternalOutput").ap()

    with tile.TileContext(nc) as tc, ExitStack() as ctx:
        singles = ctx.enter_context(tc.tile_pool(name="singles", bufs=1))
        temps = ctx.enter_context(tc.tile_pool(name="temps", bufs=2))
        psum = ctx.enter_context(tc.tile_pool(name="psum", bufs=2, space="PSUM"))
        outs = ctx.enter_context(tc.tile_pool(name="outs", bufs=4))
        dram = ctx.enter_context(tc.tile_pool(name="dram", bufs=1, space="DRAM"))

        # ---------------- constants / inputs to SBUF ----------------
        xt_sb = singles.tile([P, K_TD, Rp], F32, name="xt_sb")
        nc.sync.dma_start(out=xt_sb, in_=xt_d.rearrange("(k p) i -> p k i", p=P))
        wt_sb = singles.tile([P, K_TD, H], F32, name="wt_sb")
        nc.sync.dma_start(out=wt_sb, in_=wt_d.rearrange("(k p) h -> p k h", p=P))
        wd_sb = singles.tile([P, H], F32, name="wd_sb")
        nc.sync.dma_start(out=wd_sb, in_=wd_d)
        bt_sb = singles.tile([1, H], F32, name="bt_sb")
        nc.sync.dma_start(out=bt_sb, in_=bt_d)
        bd_sb = singles.tile([1, H], F32, name="bd_sb")
        nc.sync.dma_start(out=bd_sb, in_=bd_d)
        dft_sb = singles.tile([P, Ap], F32, name="dft_sb")
        nc.sync.dma_start(out=dft_sb, in_=dft_d)
        lhd_sb = singles.tile([5, Rp], F32, name="lhd_sb")
        nc.sync.dma_start(out=lhd_sb, in_=lhd_d)
        rhd_sb = singles.tile([5, Ap], F32, name="rhd_sb")
        nc.sync.dma_start(out=rhd_sb, in_=rhd_d)
        i4_sb = singles.tile([P, 512], F32R, name="i4_sb")
        nc.sync.dma_start(out=i4_sb, in_=i4_d)
        r2d_sb = singles.tile([Ap + 1, AH], F32R, name="r2d_sb")
        nc.sync.dma_start(out=r2d_sb[:Ap, :], in_=r2d_d[:Ap, :])

        ones_sb = singles.tile([1, 512], F32, name="ones_sb")
        nc.vector.memset(ones_sb, 1.0)
        idn = singles.tile([P, P], F32, name="idn")
        make_identity(nc, idn)

        # rows 0..Ap-1 get dnorm^T below; row Ap must stay all-ones (engines
        # can only write at 32-aligned start partitions, so memset everything)
        lhsT2 = singles.tile([Ap + 1, Rp], F32R, name="lhsT2")
        ones2 = temps.tile([Ap + 1, Rp], F32, name="ones2")
        nc.vector.memset(ones2, 1.0)
        nc.vector.tensor_copy(out=lhsT2, in_=ones2)  # f32 -> f32r rounding copy
        tfT = singles.tile([P, Rp], F32R, name="tfT")

        # ---------------- tfT = Wt.T @ X.T + bt  [H, Rp] ----------------
        for s in range(0, Rp, 512):
            w = min(512, Rp - s)
            ps_tf = psum.tile([P, 512], F32, tag="ps_small", name="ps_tf")
            for k in range(K_TD):
                nc.tensor.matmul(ps_tf[:, :w], lhsT=wt_sb[:, k, :],
                                 rhs=xt_sb[:, k, s:s + w],
                                 start=(k == 0), stop=False)
            nc.tensor.matmul(ps_tf[:, :w], lhsT=bt_sb, rhs=ones_sb[:, :w],
                             start=False, stop=True)
            nc.vector.tensor_copy(out=tfT[:, s:s + w], in_=ps_tf[:, :w])

        # ---------------- -df -> R2D row Ap ----------------
        ps_df = psum.tile([P, 512], F32, tag="ps_small", name="ps_df")
        nc.tensor.matmul(ps_df[:Ap, :H], lhsT=dft_sb, rhs=wd_sb,
                         start=True, stop=False)
        nc.tensor.matmul(ps_df[:Ap, :H], lhsT=ones_sb[:, :Ap], rhs=bd_sb,
                         start=False, stop=True)
        dfneg = temps.tile([P, H], F32R, name="dfneg")
        nc.vector.tensor_scalar_mul(dfneg[:Ap], ps_df[:Ap, :H], -1.0)
        dscr = dram.tile([AH], F32R, name="dscr")
        nc.sync.dma_start(out=dscr.rearrange("(a h) -> a h", h=H), in_=dfneg[:Ap])
        nc.sync.dma_start(out=r2d_sb[Ap:Ap + 1, :], in_=dscr[None, :])

        # ---------------- distances & per-block min/max ----------------
        rmins = temps.tile([P, RT], F32, name="rmins")
        rmaxn = temps.tile([P, RT], F32, name="rmaxn")  # -rowmax
        Dts = []
        for rt in range(RT):
            rsl = slice(P * rt, P * (rt + 1))
            ps_d = psum.tile([P, 512], F32, tag="ps_small", name="ps_d")
            nc.tensor.matmul(ps_d[:, :Ap], lhsT=lhd_sb[:, rsl], rhs=rhd_sb,
                             start=True, stop=True)
            Dt = singles.tile([P, Ap], F32, name=f"Dt{rt}")
            nc.scalar.activation(out=Dt, in_=ps_d[:, :Ap], func=AF.Sqrt)
            Dts.append(Dt)
            nc.vector.tensor_reduce(out=rmins[:, rt:rt + 1], in_=Dt,
                                    axis=AX.X, op=OP.min)
            nc.vector.tensor_reduce(out=rmaxn[:, rt:rt + 1], in_=Dt,
                                    axis=AX.X, op=OP.max, negate=True)

        stats = temps.tile([P, 2], F32, name="stats")
        nc.vector.tensor_reduce(out=stats[:, 0:1], in_=rmins, axis=AX.X, op=OP.min)
        nc.vector.tensor_reduce(out=stats[:, 1:2], in_=rmaxn, axis=AX.X, op=OP.min)
        ps_t1 = psum.tile([P, 512], F32, tag="ps_small", name="ps_t1")
        nc.tensor.transpose(ps_t1[:2, :P], stats, idn)
        # per-partition reduce: row0 -> dmin, row1 -> min(-rowmax) = -dmax
        mm2c = temps.tile([2, 1], F32, name="mm2c")
        nc.vector.tensor_reduce(out=mm2c, in_=ps_t1[:2, :P], axis=AX.X, op=OP.min)
        ps_t2 = psum.tile([P, 512], F32, tag="ps_small", name="ps_t2")
        nc.tensor.transpose(ps_t2[:1, :2], mm2c, idn[:2, :2])
        sc = temps.tile([1, 2], F32, name="sc")      # [dmin, -dmax]
        nc.vector.tensor_copy(out=sc, in_=ps_t2[:1, :2])

        diff = temps.tile([1, 1], F32, name="diff")   # dmax - dmin
        nc.vector.tensor_scalar(out=diff, in0=sc[:, 0:1], scalar1=sc[:, 1:2],
                                scalar2=-1.0, op0=OP.add, op1=OP.mult)
        denom = temps.tile([1, 1], F32, name="denom")
        nc.vector.tensor_scalar_max(denom, diff, 1e-30)
        inv = temps.tile([1, 1], F32, name="inv")
        nc.vector.reciprocal(out=inv, in_=denom)
        bv = temps.tile([1, 2], F32, name="bv")       # [dmin, 1/denom]
        nc.vector.tensor_copy(out=bv[:, 0:1], in_=sc[:, 0:1])
        nc.vector.tensor_copy(out=bv[:, 1:2], in_=inv)
        ps_b = psum.tile([P, 512], F32, tag="ps_small", name="ps_b")
        nc.tensor.matmul(ps_b[:, :2], lhsT=ones_sb[:, :P], rhs=bv,
                         start=True, stop=True)
        cols = temps.tile([P, 2], F32, name="cols")
        nc.vector.tensor_copy(out=cols, in_=ps_b[:, :2])

        # ---------------- dnorm^T into lhsT2 rows 0..Ap ----------------
        for rt in range(RT):
            rsl = slice(P * rt, P * (rt + 1))
            dn = temps.tile([P, Ap], F32, name="dn")
            nc.vector.tensor_scalar(out=dn, in0=Dts[rt],
                                    scalar1=cols[:, 0:1], scalar2=cols[:, 1:2],
                                    op0=OP.subtract, op1=OP.mult)
            ps_tt = psum.tile([P, 512], F32, tag="ps_small", name="ps_tt")
            nc.tensor.transpose(ps_tt[:Ap, :P], dn, idn)
            nc.vector.tensor_copy(out=lhsT2[:Ap, rsl], in_=ps_tt[:Ap, :P])

        # ---------------- main: psum = tf - df + dnorm ; tanh ; out ----------------
        GRP = 3  # 512-chunks per psum group (3 banks; 2 groups in flight = 6 banks)
        for rt in range(RT):
            rsl = slice(P * rt, P * (rt + 1))
            pos = 0
            while pos < NCH:
                g = min(GRP, NCH - pos)
                gw = g * 512
                pso = psum.tile([P, GRP * 512], F32, tag="ps_big", name="pso")
                for c in range(g):
                    ch = pos + c
                    csl = slice(512 * c, 512 * (c + 1))
                    nc.tensor.matmul(pso[:, csl],
                                     lhsT=tfT[:, rsl],
                                     rhs=i4_sb,
                                     start=True, stop=False)
                    nc.tensor.matmul(pso[:, csl],
                                     lhsT=lhsT2[:, rsl],
                                     rhs=r2d_sb[:, 512 * ch:512 * (ch + 1)],
                                     start=False, stop=True)
                ob = outs.tile([P, GRP * 512], F32, name="ob")
                nc.scalar.activation(out=ob[:, :gw], in_=pso[:, :gw], func=AF.Tanh)
                nc.sync.dma_start(out=out_d[rsl, 512 * pos:512 * pos + gw],
                                  in_=ob[:, :gw])
                pos += g

    nc.compile()
    return nc


_last_nc = None
_last_in_maps = None


def kernel(**inputs) -> np.ndarray:
    global _last_results, _last_nc, _last_in_maps
    in_maps, meta = _host_prep(**inputs)
    Rp, Ap = meta["Rp"], meta["Ap"]

    nc = build_bass(Rp, Ap)
    _last_nc, _last_in_maps = nc, in_maps

    from concourse.bass_utils import run_bass_kernel_spmd
    res = run_bass_kernel_spmd(nc, in_maps, core_ids=list(range(NCORES)))
    _last_results = res

    out = np.zeros((NR, NA, H), np.float32)
    for c in range(B):
        rc, ac = int(meta["r_cnt"][c]), int(meta["a_cnt"][c])
        if rc == 0 or ac == 0:
            continue
        blk = res.results[c]["out"].reshape(Rp, Ap, H)
        r0, a0 = int(meta["r0"][c]), int(meta["a0"][c])
        out[r0:r0 + rc, a0:a0 + ac, :] = blk[:rc, :ac, :]
    return out

